# revision 23
# baseline (speedup 1.0000x reference)
"""CacheAwareMHA TRN2 kernel v2: 8-core head-sharded attention, all-bf16.

Strategy (per core = 2 heads):
  - All matmuls bf16 (1 cyc/row on PE, half the DMA of fp32r); PSUM fp32.
  - Qproj streamed by k-slice: each of the 16 contraction slices of x/W_q is
    consumed as its DMA lands, killing the serial load head.
  - S^T layout [m, t]; queries sorted by position so causality is a per-m-tile
    t-suffix window. m-tiles processed in PAIRS sharing one [128,2,512] PSUM
    buffer per chunk, so exp is one wide ACT instruction per (pair, chunk).
  - Rowsum: chunk-0 (t<512, small softmax windows) via bf16 ones-matmul;
    chunk-1 (dense) via fp8e4m3 DoubleRow pair-matmul (half the PE columns)
    from an fp8 copy of P made on DVE/GpSimd. The independent fp8
    quantization only perturbs the softmax denominator (averages out).
  - Output projection bf16, partials DMA'd out as fp16, summed on host.
"""
import sys
import math

import numpy as np
import ml_dtypes

for _p in ("/opt/trn_rl_repo", "/opt/pypackages"):
    if _p not in sys.path:
        sys.path.append(_p)

T, D, H, DK, M = 1024, 2048, 16, 128, 4096
NCORES = 8
HLOC = H // NCORES  # heads per core
KO = D // 128       # 16 contraction slices for Qproj
MT = M // 128       # 32 m-tiles
NP_ = MT // 2       # 16 m-tile pairs
ROPE_BASE = 10000.0
SCALE = 1.0 / math.sqrt(DK)

USE_FP8_RS = True   # fp8 DoubleRow rowsum for the dense t>=512 half

_PROGRAM_CACHE = {}

bf16 = ml_dtypes.bfloat16
f8e4 = ml_dtypes.float8_e4m3fn


def _host_rope_k(k, pos):
    """Apply RoPE to cached keys on host (fp64 tables). k: [M, h, DK]."""
    inv = 1.0 / (ROPE_BASE ** (np.arange(0, DK, 2, dtype=np.float64) / DK))
    th = pos[:, None].astype(np.float64) * inv[None, :]
    cos = np.concatenate([np.cos(th), np.cos(th)], -1)[:, None, :]
    sin = np.concatenate([np.sin(th), np.sin(th)], -1)[:, None, :]
    t1, t2 = k[..., :64], k[..., 64:]
    rot = np.concatenate([-t2, t1], -1)
    return (k.astype(np.float64) * cos + rot.astype(np.float64) * sin).astype(np.float32)


def _host_q_tables(pos_sorted):
    """cos / sign-baked sin tables in Q^T layout [DK, T] (bf16)."""
    inv = 1.0 / (ROPE_BASE ** (np.arange(0, DK, 2, dtype=np.float64) / DK))
    th = pos_sorted[None, :].astype(np.float64) * inv[:, None]      # [64, T]
    cos = np.cos(th)
    sin = np.sin(th)
    cosT = np.concatenate([cos, cos], 0).astype(bf16)                # [128, T]
    sinT = np.concatenate([-sin, sin], 0).astype(bf16)               # sign baked
    return cosT, sinT


def _build_program(a_list, b_list):
    """Build the single-core Bass program (same for all cores)."""
    import concourse.tile as tile
    import concourse.mybir as mybir
    from concourse import bacc
    from contextlib import ExitStack

    f32 = mybir.dt.float32
    f16 = mybir.dt.float16
    bf = mybir.dt.bfloat16
    f8 = mybir.dt.float8e4

    assert max(a_list) < 512 and max(b_list) < 512  # all masking in chunk 0

    nc = bacc.Bacc("TRN2", target_bir_lowering=False, debug=False, num_devices=NCORES)

    d_xT = nc.dram_tensor("xT", (128, KO, T), bf, kind="ExternalInput").ap()
    d_wqT = nc.dram_tensor("wqT", (128, KO, HLOC * DK), bf, kind="ExternalInput").ap()
    d_ktr = nc.dram_tensor("ktr", (HLOC, DK, M), bf, kind="ExternalInput").ap()
    d_v = nc.dram_tensor("v", (HLOC, 128, MT, DK), bf, kind="ExternalInput").ap()
    d_woT = nc.dram_tensor("woT", (128, HLOC, D), bf, kind="ExternalInput").ap()
    d_cosq = nc.dram_tensor("cosq", (DK, T), bf, kind="ExternalInput").ap()
    d_sinq = nc.dram_tensor("sinq", (DK, T), bf, kind="ExternalInput").ap()
    d_bias = nc.dram_tensor("bias", (128, NP_, 2, 64), f32, kind="ExternalInput").ap()
    d_ones16 = nc.dram_tensor("ones16", (128, 128), bf, kind="ExternalInput").ap()
    d_ones8 = nc.dram_tensor("ones8", (128, 2, 128), f8, kind="ExternalInput").ap()
    d_out = nc.dram_tensor("outT", (D, T), f16, kind="ExternalOutput").ap()

    with tile.TileContext(nc) as tc, ExitStack() as ctx:
        const = ctx.enter_context(tc.tile_pool(name="const", bufs=1))
        big = ctx.enter_context(tc.tile_pool(name="big", bufs=1))
        qpool = ctx.enter_context(tc.tile_pool(name="qpool", bufs=1))
        qtmp = ctx.enter_context(tc.tile_pool(name="qtmp", bufs=2))
        ppool = ctx.enter_context(tc.tile_pool(name="ppool", bufs=6))
        p8pool = ctx.enter_context(tc.tile_pool(name="p8pool", bufs=3))
        opool = ctx.enter_context(tc.tile_pool(name="opool", bufs=1))
        ostage = ctx.enter_context(tc.tile_pool(name="ostage", bufs=4))
        ps_s = ctx.enter_context(tc.tile_pool(name="ps_s", bufs=2, space="PSUM"))
        ps_o = ctx.enter_context(tc.tile_pool(name="ps_o", bufs=1, space="PSUM"))
        ps_r = ctx.enter_context(tc.tile_pool(name="ps_r", bufs=1, space="PSUM"))
        xpool_cm = tc.tile_pool(name="xpool", bufs=1)
        xpool = xpool_cm.__enter__()

        rings = [nc.sync, nc.scalar, nc.gpsimd]

        # ---------------- k-streamed loads for Qproj ----------------
        wqT_sb = xpool.tile([128, KO, HLOC * DK], bf, name="wqT_sb")
        xT_sb = xpool.tile([128, KO, T], bf, name="xT_sb")
        xdmas = []
        for k in range(KO):
            eng = rings[k % 3]
            eng.dma_start(out=wqT_sb[:, k, :], in_=d_wqT[:, k, :])
            xdmas.append(eng.dma_start(out=xT_sb[:, k, :], in_=d_xT[:, k, :]))

        # ---------------- bulk loads for attention / output ----------------
        ring_i = 0

        def next_ring():
            nonlocal ring_i
            r = rings[ring_i % 3]
            ring_i += 1
            return r

        # rope tables first (rope is the gate after Qproj)
        cosq_sb = const.tile([128, T], bf, name="cosq_sb")
        nc.sync.dma_start(out=cosq_sb[:], in_=d_cosq)
        sinq_sb = const.tile([128, T], bf, name="sinq_sb")
        nc.scalar.dma_start(out=sinq_sb[:], in_=d_sinq)
        ones16_sb = const.tile([128, 128], bf, name="ones16_sb")
        nc.gpsimd.dma_start(out=ones16_sb[:], in_=d_ones16)
        ones8_sb = const.tile([128, 2, 128], f8, name="ones8_sb")
        nc.gpsimd.dma_start(out=ones8_sb[:], in_=d_ones8)
        # first working set: ktr/v h0 early chunks, then mask bias
        ktr_sb = [big.tile([128, M], bf, name=f"ktr_sb{h}") for h in range(HLOC)]
        v_sb = [big.tile([128, MT, DK], bf, name=f"v_sb{h}") for h in range(HLOC)]
        bias_sb = const.tile([128, NP_, 2, 64], f32, name="bias_sb")
        nc.sync.dma_start(out=ktr_sb[0][:, 0:1024], in_=d_ktr[0][:, 0:1024])
        nc.scalar.dma_start(out=v_sb[0][:, 0:8, :], in_=d_v[0][:, 0:8, :])
        for ch in range(2):
            rings[ch].dma_start(out=bias_sb[:, ch * 8:(ch + 1) * 8, :, :],
                                in_=d_bias[:, ch * 8:(ch + 1) * 8, :, :])
        nc.gpsimd.dma_start(out=ktr_sb[0][:, 1024:2048], in_=d_ktr[0][:, 1024:2048])
        for ch in range(2, 4):
            next_ring().dma_start(out=ktr_sb[0][:, ch * 1024:(ch + 1) * 1024],
                                  in_=d_ktr[0][:, ch * 1024:(ch + 1) * 1024])
        for ch in range(1, 4):
            next_ring().dma_start(out=v_sb[0][:, ch * 8:(ch + 1) * 8, :],
                                  in_=d_v[0][:, ch * 8:(ch + 1) * 8, :])
        for ch in range(4):
            next_ring().dma_start(out=ktr_sb[1][:, ch * 1024:(ch + 1) * 1024],
                                  in_=d_ktr[1][:, ch * 1024:(ch + 1) * 1024])
        for ch in range(4):
            next_ring().dma_start(out=v_sb[1][:, ch * 8:(ch + 1) * 8, :],
                                  in_=d_v[1][:, ch * 8:(ch + 1) * 8, :])
        woT_sb = big.tile([128, HLOC, D], bf, name="woT_sb")
        for ch in range(4):
            next_ring().dma_start(out=woT_sb[:, :, ch * 512:(ch + 1) * 512],
                                  in_=d_woT[:, :, ch * 512:(ch + 1) * 512])

        # ---------------- PE warmup: keep the PE at full p-state ----------------
        warm = const.tile([128, 512], bf, name="warm")
        nc.vector.memset(warm[:], 0.0)
        ps_warm = ps_r.tile([128, 512], f32, tag="r", name="warmps")
        for _ in range(40):
            nc.tensor.matmul(ps_warm[:], warm[:, 0:128], warm[:],
                             start=True, stop=True)

        # ---------------- Q projection (k-streamed) + RoPE ----------------
        qps = [
            ps_s.tile([128, T], f32, tag="s", name="qps0"),
            ps_o.tile([128, T], f32, tag="o", name="qps1"),
        ]
        qtr = [qpool.tile([128, T], bf, tag=f"qtr{h}", name=f"qtr{h}")
               for h in range(HLOC)]
        from concourse.tile_rust import add_dep_helper
        first_mm = None
        for k in range(KO):
            for h in range(HLOC):
                for c in range(2):
                    cs = slice(c * 512, (c + 1) * 512)
                    mm_ = nc.tensor.matmul(
                        qps[h][:, cs],
                        wqT_sb[:, k, h * DK:(h + 1) * DK],
                        xT_sb[:, k, cs],
                        start=(k == 0), stop=(k == KO - 1),
                    )
                    if first_mm is None:
                        first_mm = mm_
        add_dep_helper(first_mm.ins, xdmas[3].ins, sync=True,
                       reason="prefetch 3 k-slices before PE start (pstate ramp)")
        for h in range(HLOC):
            for c in range(2):
                cs = slice(c * 512, (c + 1) * 512)
                qrot = qtmp.tile([128, 512], f32, tag="qrot")
                nc.vector.tensor_copy(qrot[0:64, :], qps[h][64:128, cs])
                nc.vector.tensor_copy(qrot[64:128, :], qps[h][0:64, cs])
                t1 = qtmp.tile([128, 512], f32, tag="t1")
                nc.vector.tensor_mul(t1[:], qrot[:], sinq_sb[:, cs])
                t2 = qtmp.tile([128, 512], f32, tag="t2")
                nc.vector.tensor_mul(t2[:], qps[h][:, cs], cosq_sb[:, cs])
                nc.vector.tensor_add(qtr[h][:, cs], t1[:], t2[:])

        for _ in range(14):
            nc.tensor.matmul(ps_warm[:], warm[:, 0:128], warm[:],
                             start=True, stop=True)

        xpool_cm.__exit__(None, None, None)  # free xT/wqT SBUF

        # ---------------- attention per head, m-tile pairs ----------------
        onorm = []
        for h in range(HLOC):
            ops_t = ps_o.tile([128, T], f32, tag="o", name=f"oacc{h}")
            rs_t = ps_r.tile([128, T], f32, tag="r", name=f"rs{h}")
            pend_rs = None  # delayed fp8 rowsum matmul (software pipeline)
            for j in range(NP_):
                plo = a_list[2 * j]
                bw = min(64, 512 - plo)
                # --- S: A/B adjacent per subtile so the ktr stationary is
                # loaded once (bass skips LDWEIGHTS on unchanged stationary) ---
                spsA = ps_s.tile([128, 2, 512], f32, tag="s", name=f"sA_{h}_{j}")
                spsB = ps_s.tile([128, 2, 512], f32, tag="s", name=f"sB_{h}_{j}")
                for s_ in range(2):
                    i = 2 * j + s_
                    ksl = ktr_sb[h][:, i * 128:(i + 1) * 128]
                    nc.tensor.matmul(spsA[:, s_, plo:512], ksl,
                                     qtr[h][:, plo:512], start=True, stop=True)
                    nc.tensor.matmul(spsB[:, s_, :], ksl,
                                     qtr[h][:, 512:1024], start=True, stop=True)
                    if s_ == 0:
                        nc.vector.tensor_add(spsA[:, 0:1, plo:plo + bw],
                                             spsA[:, 0:1, plo:plo + bw],
                                             bias_sb[:, j, 0:1, 0:bw])
                nc.vector.tensor_add(spsA[:, 1:2, plo:plo + bw],
                                     spsA[:, 1:2, plo:plo + bw],
                                     bias_sb[:, j, 1:2, 0:bw])
                pA = ppool.tile([128, 2, 512], bf, tag="p")
                nc.scalar.activation(pA[:, :, plo:], spsA[:, :, plo:],
                                     mybir.ActivationFunctionType.Exp, scale=SCALE)
                pB = ppool.tile([128, 2, 512], bf, tag="p")
                nc.scalar.activation(pB[:], spsB[:],
                                     mybir.ActivationFunctionType.Exp, scale=SCALE)
                if pend_rs is not None:
                    nc.tensor.matmul(rs_t[:, 512:1024], ones8_sb[:], pend_rs[:],
                                     start=(j == 1), stop=False,
                                     perf_mode=mybir.MatmulPerfMode.DoubleRow)
                # --- P@V A/B adjacent per subtile (share the v stationary),
                # rowsum matmuls adjacent (share the ones stationary) ---
                for s_ in range(2):
                    i = 2 * j + s_
                    first = (j == 0 and s_ == 0)
                    last = (j == NP_ - 1 and s_ == 1)
                    nc.tensor.matmul(ops_t[:, plo:512], v_sb[h][:, i, :],
                                     pA[:, s_, plo:512], start=first, stop=last)
                    nc.tensor.matmul(ops_t[:, 512:1024], v_sb[h][:, i, :],
                                     pB[:, s_, :], start=first, stop=last)
                for s_ in range(2):
                    i = 2 * j + s_
                    first = (j == 0 and s_ == 0)
                    last = (j == NP_ - 1 and s_ == 1)
                    nc.tensor.matmul(rs_t[:, plo:512], ones16_sb[:],
                                     pA[:, s_, plo:512], start=first, stop=last)
                    if not USE_FP8_RS:
                        nc.tensor.matmul(rs_t[:, 512:1024], ones16_sb[:],
                                         pB[:, s_, :], start=first, stop=last)
                if USE_FP8_RS:
                    p8 = p8pool.tile([128, 2, 512], f8, tag="p8")
                    nc.vector.tensor_copy(p8[:], pB[:])
                    pend_rs = p8
            if USE_FP8_RS:
                nc.tensor.matmul(rs_t[:, 512:1024], ones8_sb[:], pend_rs[:],
                                 start=False, stop=True,
                                 perf_mode=mybir.MatmulPerfMode.DoubleRow)
            oh = opool.tile([128, T], bf, tag=f"onorm{h}", name=f"onorm{h}")
            for c in range(2):
                cs = slice(c * 512, (c + 1) * 512)
                rsinv = qtmp.tile([128, 512], f32, tag="rsinv")
                nc.vector.reciprocal_approx_fast(out=rsinv[:], in_=rs_t[:, cs])
                nc.vector.tensor_mul(oh[:, cs], ops_t[:, cs], rsinv[:])
            onorm.append(oh)

        # ---------------- output projection ----------------
        outT_r = d_out.rearrange("(jo p) t -> p jo t", p=128)
        orings = [nc.sync, nc.scalar, nc.gpsimd]
        jrot = [(ps_s, "s"), (ps_o, "o"), (ps_s, "s"), (ps_r, "r")]
        for j in range(KO):
            jpool, jtag = jrot[j % 4]
            jps = jpool.tile([128, T], f32, tag=jtag, name=f"jps{j}")
            for ho in range(HLOC):
                for c in range(2):
                    cs = slice(c * 512, (c + 1) * 512)
                    nc.tensor.matmul(
                        jps[:, cs],
                        woT_sb[:, ho, j * 128:(j + 1) * 128],
                        onorm[ho][:, cs],
                        start=(ho == 0), stop=(ho == HLOC - 1),
                    )
            ost = ostage.tile([128, T], f16, tag="ost")
            nc.vector.tensor_copy(ost[:, 0:512], jps[:, 0:512])
            nc.scalar.copy(ost[:, 512:1024], jps[:, 512:1024])
            orings[j % 3].dma_start(out=outT_r[:, j, :], in_=ost[:])

    nc.compile()
    return nc


def _prep(inputs):
    """Host-side prep shared by kernel() and test harnesses."""
    x = np.asarray(inputs["x"], dtype=np.float32)
    k_ctx = np.asarray(inputs["k_ctx"], dtype=np.float32)
    v_ctx = np.asarray(inputs["v_ctx"], dtype=np.float32)
    W_q = np.asarray(inputs["W_q"], dtype=np.float32)
    W_o = np.asarray(inputs["W_o"], dtype=np.float32)
    pos_np = np.asarray(inputs["positions"]).astype(np.int64)
    pctx_np = np.asarray(inputs["p_ctx"]).astype(np.int64)

    perm = np.argsort(pos_np, kind="stable")
    ps = pos_np[perm]
    xT = np.ascontiguousarray(
        x[perm].T.reshape(KO, 128, T).transpose(1, 0, 2)).astype(bf16)
    k_rope = _host_rope_k(k_ctx, pctx_np)
    cosq, sinq = _host_q_tables(ps)
    # additive causal-mask bias: for pair j, band cols [a_2j, a_2j+bw);
    # bias[m, j, s, c] = -1e5 where pos_{t=a_2j+c} < (2j+s)*128 + m else 0
    bias = np.zeros((128, NP_, 2, 64), dtype=np.float32)
    a_l = [int(np.searchsorted(ps, 128 * i, side="left")) for i in range(MT)]
    b_l = [int(np.searchsorted(ps, 128 * i + 127, side="left")) for i in range(MT)]
    for j in range(NP_):
        plo = a_l[2 * j]
        bw = min(64, 512 - plo)
        assert b_l[2 * j + 1] <= plo + bw, "mask band exceeds 64-col window"
        tcols = ps[plo:plo + bw]                                  # [bw]
        for s_ in range(2):
            mrow = (2 * j + s_) * 128 + np.arange(128)            # [128]
            bias[:, j, s_, :bw] = np.where(
                tcols[None, :] >= mrow[:, None], 0.0, -1e5)
    ones16 = np.ones((128, 128), dtype=bf16)
    ones8 = np.ones((128, 2, 128), dtype=f8e4)
    a_list = [int(np.searchsorted(ps, 128 * i, side="left")) for i in range(MT)]
    b_list = [int(np.searchsorted(ps, 128 * i + 127, side="left")) for i in range(MT)]

    in_maps = []
    for c in range(NCORES):
        hs = slice(c * HLOC * DK, (c + 1) * HLOC * DK)
        heads = range(c * HLOC, (c + 1) * HLOC)
        wq = W_q[hs, :].T.reshape(KO, 128, HLOC * DK)          # [ko, p, o]
        wo = W_o[:, hs].T.reshape(HLOC, 128, D)                 # [ho, p, j]
        vv = v_ctx.transpose(1, 0, 2)[c * HLOC:(c + 1) * HLOC]  # [hloc, M, DK]
        in_maps.append({
            "xT": xT,
            "wqT": np.ascontiguousarray(wq.transpose(1, 0, 2)).astype(bf16),
            "ktr": np.ascontiguousarray(
                np.stack([k_rope[:, h, :].T for h in heads])).astype(bf16),
            "v": np.ascontiguousarray(
                vv.reshape(HLOC, MT, 128, DK).transpose(0, 2, 1, 3)).astype(bf16),
            "woT": np.ascontiguousarray(wo.transpose(1, 0, 2)).astype(bf16),
            "cosq": cosq, "sinq": sinq, "bias": bias,
            "ones16": ones16, "ones8": ones8,
        })
    return perm, a_list, b_list, in_maps


def kernel(x, k_ctx, v_ctx, W_q, W_o, positions, p_ctx):
    from concourse.bass_utils import run_bass_kernel_spmd

    inputs = dict(x=x, k_ctx=k_ctx, v_ctx=v_ctx, W_q=W_q, W_o=W_o,
                  positions=positions, p_ctx=p_ctx)
    perm, a_list, b_list, in_maps = _prep(inputs)

    key = (tuple(a_list), tuple(b_list))
    if key not in _PROGRAM_CACHE:
        _PROGRAM_CACHE[key] = _build_program(a_list, b_list)
    nc = _PROGRAM_CACHE[key]

    r = run_bass_kernel_spmd(nc, in_maps, core_ids=list(range(NCORES)))

    acc = np.zeros((D, T), dtype=np.float64)
    for c in range(NCORES):
        acc += r.results[c]["outT"].astype(np.float64)
    out_sorted = acc.T.astype(np.float32)
    out = np.empty_like(out_sorted)
    out[perm] = out_sorted
    return out.astype(np.float32)


if __name__ == "__main__":
    import importlib.util
    spec = importlib.util.spec_from_file_location("reference", "/root/problem/reference.py")
    ref = importlib.util.module_from_spec(spec)
    ref_mod = importlib.util.module_from_spec(spec)
    spec.loader.exec_module(ref)
    inputs = {k: np.asarray(v) for k, v in ref.setup_inputs().items()}
    expected = np.asarray(ref.reference(**inputs))
    got = kernel(**inputs)
    err = np.abs(got - expected)
    print("absmax err:", err.max(), "rel:", err.max() / np.abs(expected).max())


# revision 25
# speedup vs baseline: 1.0592x; 1.0592x over previous
"""CacheAwareMHA TRN2 kernel v2: 8-core head-sharded attention, all-bf16.

Strategy (per core = 2 heads):
  - All matmuls bf16 (1 cyc/row on PE, half the DMA of fp32r); PSUM fp32.
  - Qproj streamed by k-slice: each of the 16 contraction slices of x/W_q is
    consumed as its DMA lands, killing the serial load head.
  - S^T layout [m, t]; queries sorted by position so causality is a per-m-tile
    t-suffix window. m-tiles processed in PAIRS sharing one [128,2,512] PSUM
    buffer per chunk, so exp is one wide ACT instruction per (pair, chunk).
  - Rowsum: chunk-0 (t<512, small softmax windows) via bf16 ones-matmul;
    chunk-1 (dense) via fp8e4m3 DoubleRow pair-matmul (half the PE columns)
    from an fp8 copy of P made on DVE/GpSimd. The independent fp8
    quantization only perturbs the softmax denominator (averages out).
  - Output projection bf16, partials DMA'd out as fp16, summed on host.
"""
import sys
import math

import numpy as np
import ml_dtypes

for _p in ("/opt/trn_rl_repo", "/opt/pypackages"):
    if _p not in sys.path:
        sys.path.append(_p)

T, D, H, DK, M = 1024, 2048, 16, 128, 4096
NCORES = 8
HLOC = H // NCORES  # heads per core
KO = D // 128       # 16 contraction slices for Qproj
MT = M // 128       # 32 m-tiles
NP_ = MT // 2       # 16 m-tile pairs
ROPE_BASE = 10000.0
SCALE = 1.0 / math.sqrt(DK)

USE_FP8_RS = True   # fp8 DoubleRow rowsum for the dense t>=512 half

_PROGRAM_CACHE = {}

bf16 = ml_dtypes.bfloat16
f8e4 = ml_dtypes.float8_e4m3fn


def _host_rope_k(k, pos):
    """Apply RoPE to cached keys on host (fp64 tables). k: [M, h, DK]."""
    inv = 1.0 / (ROPE_BASE ** (np.arange(0, DK, 2, dtype=np.float64) / DK))
    th = pos[:, None].astype(np.float64) * inv[None, :]
    cos = np.concatenate([np.cos(th), np.cos(th)], -1)[:, None, :]
    sin = np.concatenate([np.sin(th), np.sin(th)], -1)[:, None, :]
    t1, t2 = k[..., :64], k[..., 64:]
    rot = np.concatenate([-t2, t1], -1)
    return (k.astype(np.float64) * cos + rot.astype(np.float64) * sin).astype(np.float32)


def _host_q_tables(pos_sorted):
    """cos / sign-baked sin tables in Q^T layout [DK, T] (bf16)."""
    inv = 1.0 / (ROPE_BASE ** (np.arange(0, DK, 2, dtype=np.float64) / DK))
    th = pos_sorted[None, :].astype(np.float64) * inv[:, None]      # [64, T]
    cos = np.cos(th)
    sin = np.sin(th)
    cosT = np.concatenate([cos, cos], 0).astype(bf16)                # [128, T]
    sinT = np.concatenate([-sin, sin], 0).astype(bf16)               # sign baked
    return cosT, sinT


def _build_program(a_list, b_list):
    """Build the single-core Bass program (same for all cores)."""
    import concourse.tile as tile
    import concourse.mybir as mybir
    from concourse import bacc
    from contextlib import ExitStack

    f32 = mybir.dt.float32
    f16 = mybir.dt.float16
    bf = mybir.dt.bfloat16
    f8 = mybir.dt.float8e4

    assert max(a_list) < 512 and max(b_list) < 512  # all masking in chunk 0

    nc = bacc.Bacc("TRN2", target_bir_lowering=False, debug=False, num_devices=NCORES)

    d_xT = nc.dram_tensor("xT", (128, KO, T), bf, kind="ExternalInput").ap()
    d_wqT = nc.dram_tensor("wqT", (128, KO, HLOC * DK), bf, kind="ExternalInput").ap()
    d_ktr = nc.dram_tensor("ktr", (HLOC, DK, M), bf, kind="ExternalInput").ap()
    d_v = nc.dram_tensor("v", (HLOC, 128, MT, DK), bf, kind="ExternalInput").ap()
    d_woT = nc.dram_tensor("woT", (128, HLOC, D), bf, kind="ExternalInput").ap()
    d_cosq = nc.dram_tensor("cosq", (DK, T), bf, kind="ExternalInput").ap()
    d_sinq = nc.dram_tensor("sinq", (DK, T), bf, kind="ExternalInput").ap()
    d_bias = nc.dram_tensor("bias", (128, NP_, 2, 64), f32, kind="ExternalInput").ap()
    d_ones16 = nc.dram_tensor("ones16", (128, 128), bf, kind="ExternalInput").ap()
    d_ones8 = nc.dram_tensor("ones8", (128, 2, 128), f8, kind="ExternalInput").ap()
    d_out = nc.dram_tensor("outT", (D, T), f16, kind="ExternalOutput").ap()

    with tile.TileContext(nc) as tc, ExitStack() as ctx:
        const = ctx.enter_context(tc.tile_pool(name="const", bufs=1))
        big = ctx.enter_context(tc.tile_pool(name="big", bufs=1))
        qpool = ctx.enter_context(tc.tile_pool(name="qpool", bufs=1))
        qtmp = ctx.enter_context(tc.tile_pool(name="qtmp", bufs=2))
        ppool = ctx.enter_context(tc.tile_pool(name="ppool", bufs=6))
        p8pool = ctx.enter_context(tc.tile_pool(name="p8pool", bufs=3))
        opool = ctx.enter_context(tc.tile_pool(name="opool", bufs=1))
        ostage = ctx.enter_context(tc.tile_pool(name="ostage", bufs=4))
        ps_s = ctx.enter_context(tc.tile_pool(name="ps_s", bufs=2, space="PSUM"))
        ps_o = ctx.enter_context(tc.tile_pool(name="ps_o", bufs=1, space="PSUM"))
        ps_r = ctx.enter_context(tc.tile_pool(name="ps_r", bufs=1, space="PSUM"))
        xpool_cm = tc.tile_pool(name="xpool", bufs=1)
        xpool = xpool_cm.__enter__()

        rings = [nc.sync, nc.scalar, nc.gpsimd]

        # ---------------- k-streamed loads for Qproj ----------------
        wqT_sb = xpool.tile([128, KO, HLOC * DK], bf, name="wqT_sb")
        xT_sb = xpool.tile([128, KO, T], bf, name="xT_sb")
        xdmas = []
        for k in range(KO):
            eng = rings[k % 3]
            eng.dma_start(out=wqT_sb[:, k, :], in_=d_wqT[:, k, :])
            xdmas.append(eng.dma_start(out=xT_sb[:, k, :], in_=d_xT[:, k, :]))

        # ---------------- bulk loads for attention / output ----------------
        ring_i = 0

        def next_ring():
            nonlocal ring_i
            r = rings[ring_i % 3]
            ring_i += 1
            return r

        # rope tables first (rope is the gate after Qproj)
        cosq_sb = const.tile([128, T], bf, name="cosq_sb")
        nc.sync.dma_start(out=cosq_sb[:], in_=d_cosq)
        sinq_sb = const.tile([128, T], bf, name="sinq_sb")
        nc.scalar.dma_start(out=sinq_sb[:], in_=d_sinq)
        ones16_sb = const.tile([128, 128], bf, name="ones16_sb")
        nc.gpsimd.dma_start(out=ones16_sb[:], in_=d_ones16)
        ones8_sb = const.tile([128, 2, 128], f8, name="ones8_sb")
        nc.gpsimd.dma_start(out=ones8_sb[:], in_=d_ones8)
        # first working set: ktr/v h0 early chunks, then mask bias
        ktr_sb = [big.tile([128, M], bf, name=f"ktr_sb{h}") for h in range(HLOC)]
        v_sb = [big.tile([128, MT, DK], bf, name=f"v_sb{h}") for h in range(HLOC)]
        bias_sb = const.tile([128, NP_, 2, 64], f32, name="bias_sb")
        nc.sync.dma_start(out=ktr_sb[0][:, 0:256], in_=d_ktr[0][:, 0:256])
        nc.sync.dma_start(out=ktr_sb[0][:, 256:1024], in_=d_ktr[0][:, 256:1024])
        nc.scalar.dma_start(out=v_sb[0][:, 0:8, :], in_=d_v[0][:, 0:8, :])
        for ch in range(2):
            rings[ch].dma_start(out=bias_sb[:, ch * 8:(ch + 1) * 8, :, :],
                                in_=d_bias[:, ch * 8:(ch + 1) * 8, :, :])
        nc.gpsimd.dma_start(out=ktr_sb[0][:, 1024:2048], in_=d_ktr[0][:, 1024:2048])
        for ch in range(2, 4):
            next_ring().dma_start(out=ktr_sb[0][:, ch * 1024:(ch + 1) * 1024],
                                  in_=d_ktr[0][:, ch * 1024:(ch + 1) * 1024])
        for ch in range(1, 4):
            next_ring().dma_start(out=v_sb[0][:, ch * 8:(ch + 1) * 8, :],
                                  in_=d_v[0][:, ch * 8:(ch + 1) * 8, :])
        for ch in range(4):
            next_ring().dma_start(out=ktr_sb[1][:, ch * 1024:(ch + 1) * 1024],
                                  in_=d_ktr[1][:, ch * 1024:(ch + 1) * 1024])
        for ch in range(4):
            next_ring().dma_start(out=v_sb[1][:, ch * 8:(ch + 1) * 8, :],
                                  in_=d_v[1][:, ch * 8:(ch + 1) * 8, :])
        woT_sb = big.tile([128, HLOC, D], bf, name="woT_sb")
        for ch in range(4):
            next_ring().dma_start(out=woT_sb[:, :, ch * 512:(ch + 1) * 512],
                                  in_=d_woT[:, :, ch * 512:(ch + 1) * 512])

        # ---------------- PE warmup: keep the PE at full p-state ----------------
        warm = const.tile([128, 512], bf, name="warm")
        nc.vector.memset(warm[:], 0.0)
        ps_warm = ps_r.tile([128, 512], f32, tag="r", name="warmps")
        for _ in range(40):
            nc.tensor.matmul(ps_warm[:], warm[:, 0:128], warm[:],
                             start=True, stop=True)

        # ---------------- Q projection (k-streamed) + RoPE ----------------
        qps = [
            ps_s.tile([128, T], f32, tag="s", name="qps0"),
            ps_o.tile([128, T], f32, tag="o", name="qps1"),
        ]
        qtr = [qpool.tile([128, T], bf, tag=f"qtr{h}", name=f"qtr{h}")
               for h in range(HLOC)]
        from concourse.tile_rust import add_dep_helper
        first_mm = None
        for k in range(KO):
            for h in range(HLOC):
                for c in range(2):
                    cs = slice(c * 512, (c + 1) * 512)
                    mm_ = nc.tensor.matmul(
                        qps[h][:, cs],
                        wqT_sb[:, k, h * DK:(h + 1) * DK],
                        xT_sb[:, k, cs],
                        start=(k == 0), stop=(k == KO - 1),
                    )
                    if first_mm is None:
                        first_mm = mm_
        add_dep_helper(first_mm.ins, xdmas[3].ins, sync=True,
                       reason="prefetch 3 k-slices before PE start (pstate ramp)")
        for h in range(HLOC):
            for c in range(2):
                cs = slice(c * 512, (c + 1) * 512)
                qrot = qtmp.tile([128, 512], f32, tag="qrot")
                nc.vector.tensor_copy(qrot[0:64, :], qps[h][64:128, cs])
                nc.vector.tensor_copy(qrot[64:128, :], qps[h][0:64, cs])
                t1 = qtmp.tile([128, 512], f32, tag="t1")
                nc.vector.tensor_mul(t1[:], qrot[:], sinq_sb[:, cs])
                t2 = qtmp.tile([128, 512], f32, tag="t2")
                nc.vector.tensor_mul(t2[:], qps[h][:, cs], cosq_sb[:, cs])
                nc.vector.tensor_add(qtr[h][:, cs], t1[:], t2[:])

        for _ in range(20):
            nc.tensor.matmul(ps_warm[:], warm[:, 0:128], warm[:],
                             start=True, stop=True)

        xpool_cm.__exit__(None, None, None)  # free xT/wqT SBUF

        # ---------------- attention per head, m-tile pairs ----------------
        onorm = []
        for h in range(HLOC):
            ops_t = ps_o.tile([128, T], f32, tag="o", name=f"oacc{h}")
            rs_t = ps_r.tile([128, T], f32, tag="r", name=f"rs{h}")
            pend_rs = None  # delayed fp8 rowsum matmul (software pipeline)
            for j in range(NP_):
                plo = a_list[2 * j]
                bw = min(64, 512 - plo)
                # --- S chunk 0, mask bias, exp A (early) ---
                spsA = ps_s.tile([128, 2, 512], f32, tag="s", name=f"sA_{h}_{j}")
                spsB = ps_s.tile([128, 2, 512], f32, tag="s", name=f"sB_{h}_{j}")
                for s_ in range(2):
                    i = 2 * j + s_
                    ksl = ktr_sb[h][:, i * 128:(i + 1) * 128]
                    nc.tensor.matmul(spsA[:, s_, plo:512], ksl,
                                     qtr[h][:, plo:512], start=True, stop=True)
                nc.vector.tensor_add(spsA[:, :, plo:plo + bw],
                                     spsA[:, :, plo:plo + bw],
                                     bias_sb[:, j, :, 0:bw])
                pA = ppool.tile([128, 2, 512], bf, tag="p")
                nc.scalar.activation(pA[:, :, plo:], spsA[:, :, plo:],
                                     mybir.ActivationFunctionType.Exp, scale=SCALE)
                # --- S chunk 1, exp B (PE streams while ACT does exp A) ---
                for s_ in range(2):
                    i = 2 * j + s_
                    ksl = ktr_sb[h][:, i * 128:(i + 1) * 128]
                    nc.tensor.matmul(spsB[:, s_, :], ksl,
                                     qtr[h][:, 512:1024], start=True, stop=True)
                pB = ppool.tile([128, 2, 512], bf, tag="p")
                nc.scalar.activation(pB[:], spsB[:],
                                     mybir.ActivationFunctionType.Exp, scale=SCALE)
                if pend_rs is not None:
                    nc.tensor.matmul(rs_t[:, 512:1024], ones8_sb[:], pend_rs[:],
                                     start=(j == 1), stop=False,
                                     perf_mode=mybir.MatmulPerfMode.DoubleRow)
                # --- P@V + rowsum: A-gated block first, then B-gated ---
                for s_ in range(2):
                    i = 2 * j + s_
                    first = (j == 0 and s_ == 0)
                    last = (j == NP_ - 1 and s_ == 1)
                    nc.tensor.matmul(ops_t[:, plo:512], v_sb[h][:, i, :],
                                     pA[:, s_, plo:512], start=first, stop=last)
                    nc.tensor.matmul(rs_t[:, plo:512], ones16_sb[:],
                                     pA[:, s_, plo:512], start=first, stop=last)
                for s_ in range(2):
                    i = 2 * j + s_
                    first = (j == 0 and s_ == 0)
                    last = (j == NP_ - 1 and s_ == 1)
                    nc.tensor.matmul(ops_t[:, 512:1024], v_sb[h][:, i, :],
                                     pB[:, s_, :], start=first, stop=last)
                    if not USE_FP8_RS:
                        nc.tensor.matmul(rs_t[:, 512:1024], ones16_sb[:],
                                         pB[:, s_, :], start=first, stop=last)
                if USE_FP8_RS:
                    p8 = p8pool.tile([128, 2, 512], f8, tag="p8")
                    nc.vector.tensor_copy(p8[:], pB[:])
                    pend_rs = p8
            if USE_FP8_RS:
                nc.tensor.matmul(rs_t[:, 512:1024], ones8_sb[:], pend_rs[:],
                                 start=False, stop=True,
                                 perf_mode=mybir.MatmulPerfMode.DoubleRow)
            oh = opool.tile([128, T], bf, tag=f"onorm{h}", name=f"onorm{h}")
            for c in range(2):
                cs = slice(c * 512, (c + 1) * 512)
                rsinv = qtmp.tile([128, 512], f32, tag="rsinv")
                nc.vector.reciprocal_approx_fast(out=rsinv[:], in_=rs_t[:, cs])
                nc.vector.tensor_mul(oh[:, cs], ops_t[:, cs], rsinv[:])
            onorm.append(oh)

        # ---------------- output projection ----------------
        outT_r = d_out.rearrange("(jo p) t -> p jo t", p=128)
        orings = [nc.sync, nc.scalar, nc.gpsimd]
        jrot = [(ps_s, "s"), (ps_o, "o"), (ps_s, "s"), (ps_r, "r")]
        for j in range(KO):
            jpool, jtag = jrot[j % 4]
            jps = jpool.tile([128, T], f32, tag=jtag, name=f"jps{j}")
            for ho in range(HLOC):
                for c in range(2):
                    cs = slice(c * 512, (c + 1) * 512)
                    nc.tensor.matmul(
                        jps[:, cs],
                        woT_sb[:, ho, j * 128:(j + 1) * 128],
                        onorm[ho][:, cs],
                        start=(ho == 0), stop=(ho == HLOC - 1),
                    )
            ost = ostage.tile([128, T], f16, tag="ost")
            nc.vector.tensor_copy(ost[:, 0:512], jps[:, 0:512])
            nc.scalar.copy(ost[:, 512:1024], jps[:, 512:1024])
            orings[j % 3].dma_start(out=outT_r[:, j, 0:512], in_=ost[:, 0:512])
            orings[(j + 1) % 3].dma_start(out=outT_r[:, j, 512:1024],
                                          in_=ost[:, 512:1024])

    nc.compile()
    return nc


def _prep(inputs):
    """Host-side prep shared by kernel() and test harnesses."""
    x = np.asarray(inputs["x"], dtype=np.float32)
    k_ctx = np.asarray(inputs["k_ctx"], dtype=np.float32)
    v_ctx = np.asarray(inputs["v_ctx"], dtype=np.float32)
    W_q = np.asarray(inputs["W_q"], dtype=np.float32)
    W_o = np.asarray(inputs["W_o"], dtype=np.float32)
    pos_np = np.asarray(inputs["positions"]).astype(np.int64)
    pctx_np = np.asarray(inputs["p_ctx"]).astype(np.int64)

    perm = np.argsort(pos_np, kind="stable")
    ps = pos_np[perm]
    xT = np.ascontiguousarray(
        x[perm].T.reshape(KO, 128, T).transpose(1, 0, 2)).astype(bf16)
    k_rope = _host_rope_k(k_ctx, pctx_np)
    cosq, sinq = _host_q_tables(ps)
    # additive causal-mask bias: for pair j, band cols [a_2j, a_2j+bw);
    # bias[m, j, s, c] = -1e5 where pos_{t=a_2j+c} < (2j+s)*128 + m else 0
    bias = np.zeros((128, NP_, 2, 64), dtype=np.float32)
    a_l = [int(np.searchsorted(ps, 128 * i, side="left")) for i in range(MT)]
    b_l = [int(np.searchsorted(ps, 128 * i + 127, side="left")) for i in range(MT)]
    for j in range(NP_):
        plo = a_l[2 * j]
        bw = min(64, 512 - plo)
        assert b_l[2 * j + 1] <= plo + bw, "mask band exceeds 64-col window"
        tcols = ps[plo:plo + bw]                                  # [bw]
        for s_ in range(2):
            mrow = (2 * j + s_) * 128 + np.arange(128)            # [128]
            bias[:, j, s_, :bw] = np.where(
                tcols[None, :] >= mrow[:, None], 0.0, -1e5)
    ones16 = np.ones((128, 128), dtype=bf16)
    ones8 = np.ones((128, 2, 128), dtype=f8e4)
    a_list = [int(np.searchsorted(ps, 128 * i, side="left")) for i in range(MT)]
    b_list = [int(np.searchsorted(ps, 128 * i + 127, side="left")) for i in range(MT)]

    in_maps = []
    for c in range(NCORES):
        hs = slice(c * HLOC * DK, (c + 1) * HLOC * DK)
        heads = range(c * HLOC, (c + 1) * HLOC)
        wq = W_q[hs, :].T.reshape(KO, 128, HLOC * DK)          # [ko, p, o]
        wo = W_o[:, hs].T.reshape(HLOC, 128, D)                 # [ho, p, j]
        vv = v_ctx.transpose(1, 0, 2)[c * HLOC:(c + 1) * HLOC]  # [hloc, M, DK]
        in_maps.append({
            "xT": xT,
            "wqT": np.ascontiguousarray(wq.transpose(1, 0, 2)).astype(bf16),
            "ktr": np.ascontiguousarray(
                np.stack([k_rope[:, h, :].T for h in heads])).astype(bf16),
            "v": np.ascontiguousarray(
                vv.reshape(HLOC, MT, 128, DK).transpose(0, 2, 1, 3)).astype(bf16),
            "woT": np.ascontiguousarray(wo.transpose(1, 0, 2)).astype(bf16),
            "cosq": cosq, "sinq": sinq, "bias": bias,
            "ones16": ones16, "ones8": ones8,
        })
    return perm, a_list, b_list, in_maps


def kernel(x, k_ctx, v_ctx, W_q, W_o, positions, p_ctx):
    from concourse.bass_utils import run_bass_kernel_spmd

    inputs = dict(x=x, k_ctx=k_ctx, v_ctx=v_ctx, W_q=W_q, W_o=W_o,
                  positions=positions, p_ctx=p_ctx)
    perm, a_list, b_list, in_maps = _prep(inputs)

    key = (tuple(a_list), tuple(b_list))
    if key not in _PROGRAM_CACHE:
        _PROGRAM_CACHE[key] = _build_program(a_list, b_list)
    nc = _PROGRAM_CACHE[key]

    r = run_bass_kernel_spmd(nc, in_maps, core_ids=list(range(NCORES)))

    acc = np.zeros((D, T), dtype=np.float64)
    for c in range(NCORES):
        acc += r.results[c]["outT"].astype(np.float64)
    out_sorted = acc.T.astype(np.float32)
    out = np.empty_like(out_sorted)
    out[perm] = out_sorted
    return out.astype(np.float32)


if __name__ == "__main__":
    import importlib.util
    spec = importlib.util.spec_from_file_location("reference", "/root/problem/reference.py")
    ref = importlib.util.module_from_spec(spec)
    ref_mod = importlib.util.module_from_spec(spec)
    spec.loader.exec_module(ref)
    inputs = {k: np.asarray(v) for k, v in ref.setup_inputs().items()}
    expected = np.asarray(ref.reference(**inputs))
    got = kernel(**inputs)
    err = np.abs(got - expected)
    print("absmax err:", err.max(), "rel:", err.max() / np.abs(expected).max())
